# revision 40
# baseline (speedup 1.0000x reference)
"""MoE FFN (top-2 of 8 experts) Trainium2 kernel.

Strategy (expert-parallel across 8 NeuronCores):
  - Host computes the (tiny) router: logits = x@Wg, softmax, top-2,
    renormalized combine weights.  Tokens are gathered per expert on the
    host ("all-to-all dispatch" done at sharding time), transposed to
    [H, C] so both FFN GEMMs run with natural weight layouts on device.
  - Core e runs the FFN for expert e over its C_pad gathered tokens.
  - GEMM1 is dense: plain W1 f-slices stream from HBM in 4-f-tile
    batches, alternating between the two DMA queues (one queue alone
    peaks well under the ~320 GB/s per-core cap).
  - GEMM2 uses one level of the Winograd variant of Strassen (7
    products, 15 additions) over the 2x2x2 split of [Nt,F]x[F,H]: the
    seven moving-side W2 operands (B11, B21, B22, T1..T4) are host-
    precomputed, loaded ONCE at the head in first-use order, and stay
    resident (112 KiB/partition); the four stationary-side h combos
    (S1..S4) trickle out on the vector engine as GEMM1 activations
    land (no pipeline seam); the C-block recombination runs as 7
    vector-engine ops (one PSUM operand each) plus 2 scalar Identity
    copies per token-tile.  Cuts GEMM2's PE row count by 12.5%.
  - Host applies combine weights + b2 and scatter-adds back ("combine").

The kernel is compiled once per (C_pad, biases-zero) configuration and
cached in-process; C_pad is rounded up to a multiple of 512.
"""

import os
import sys
import numpy as np

for _p in ("/opt/trn_rl_repo", "/root/.axon_site/_ro/trn_rl_repo"):
    if _p not in sys.path and os.path.isdir(_p):
        sys.path.append(_p)

import concourse.bacc as bacc  # noqa: E402
import concourse.tile as tile  # noqa: E402
from concourse import mybir  # noqa: E402
from concourse.bass_utils import run_bass_kernel_spmd  # noqa: E402

# Problem shapes (hardcoded per spec)
B, S, H, F, E = 4, 2048, 1024, 4096, 8
T = B * S
TOP_K = 2
N_CORES = 8
P = 128
KH = H // P          # 8  H-contraction subtiles
FT = F // P          # 32 f-tiles total
FH = FT // 2         # 16 f-tiles per Winograd half

F32 = mybir.dt.float32
BF16 = mybir.dt.bfloat16
ADD = mybir.AluOpType.add
SUB = mybir.AluOpType.subtract

_CACHE: dict = {}
LAST_RESULT = None  # BassKernelResults of the most recent run (for test.py)


def _build(n512: int, use_b1: bool):
    nrows = n512 * 4

    nc = bacc.Bacc(
        "TRN2",
        target_bir_lowering=False,
        debug=False,
        enable_asserts=False,
        num_devices=N_CORES,
    )

    # x staged chunk-major: each chunk is contiguous per partition.
    xda = nc.dram_tensor("xda", [P, n512, KH, 512], BF16, kind="ExternalInput").ap()
    # plain W1 tiles, streamed per 4-f-slice batch for the dense GEMM1
    w1d = nc.dram_tensor("w1d", [P, FT, KH, P], BF16, kind="ExternalInput").ap()
    # Winograd moving-side W2 operands: plain quarters and T-combos.
    # w2qd ops = (B11, B21, B22); w2Td ops = (T1, T2, T3, T4), each
    # [p, op, k2', n] with k2' the F-subtile within the F-half.
    w2qd = nc.dram_tensor("w2qd", [P, 3, FH, 512], BF16, kind="ExternalInput").ap()
    w2xd = nc.dram_tensor("w2xd", [P, FH, 512], BF16, kind="ExternalInput").ap()
    if use_b1:
        b1d = nc.dram_tensor("b1d", [P, FT], F32, kind="ExternalInput").ap()
    yd = nc.dram_tensor("yd", [P, nrows, H], BF16, kind="ExternalOutput").ap()

    gelu = mybir.ActivationFunctionType.Gelu_apprx_tanh
    ident = mybir.ActivationFunctionType.Identity

    with tile.TileContext(nc) as tc:
        with (
            tc.tile_pool(name="xp", bufs=1) as xp,
            tc.tile_pool(name="w1sp", bufs=2) as w1sp,
            tc.tile_pool(name="hp", bufs=1) as hp,
            tc.tile_pool(name="up", bufs=1) as up,
            tc.tile_pool(name="w2qp", bufs=1) as w2qp,
            tc.tile_pool(name="w2Tp", bufs=1) as w2Tp,
            tc.tile_pool(name="srp", bufs=2) as srp,
            tc.tile_pool(name="op", bufs=3) as op,
            tc.tile_pool(name="bp", bufs=1) as bp,
            tc.tile_pool(name="pp", bufs=8, space="PSUM") as pp,
        ):
            if use_b1:
                b1t = bp.tile([P, FT], F32)
                nc.sync.dma_start(b1t[:], b1d[:])

            # HAM warmup: the PE clock-gate sits at 1.2 GHz until it sees
            # ~3.4us of sustained matmul activity.  The PE is idle anyway
            # while the first operands stream in; burn that window on junk
            # N=64 matmuls so the real stream starts at the full 2.4 GHz.
            wub = op.tile([P, 512], BF16, tag="ot")
            nc.gpsimd.memset(wub[:, :P], 0)
            wups = pp.tile([P, 512], F32, tag="pp")
            for _ in range(170):
                nc.tensor.matmul(wups[:, :64], wub[:, :P], wub[:, :64], start=True, stop=True)

            # Chunk-0 x at the FRONT of the sync ring, then the W2-side
            # Winograd operands in first-use order (their deadlines are
            # staggered through the first chunk's GEMM2 product sweep).
            xt0 = xp.tile([P, KH, 512], BF16, tag="xt")
            nc.sync.dma_start(xt0[:, :4], xda[:, 0, :4])
            nc.sync.dma_start(xt0[:, 4:], xda[:, 0, 4:])

            w2q = w2qp.tile([P, 3, FH, 512], BF16)
            w2T = w2Tp.tile([P, 4, FH, 512], BF16)

            # h-side Winograd combos, trickled during GEMM1:
            #   U1=A21+A22, U2=U1-A11, U3=A11-A21, U4=A12-U2
            ut = up.tile([P, 4, FH, 256], BF16)

            coff = 0
            xtn = None
            for ci in range(n512):
                xt = xt0 if ci == 0 else xtn

                # ---- GEMM1, dense, W1 batches alternating DMA queues ----
                hq = hp.tile([P, FT, 512], BF16, tag="hq")
                for f in range(FT):
                    if f % 4 == 0:
                        wst = w1sp.tile([P, 4, KH, P], BF16, tag="wst")
                        if ci == 0:
                            eng = nc.gpsimd if f <= 12 else nc.sync
                        else:
                            eng = nc.gpsimd if (f // 4) % 2 == 0 else nc.sync
                        if ci == 0 and f == 0:
                            # halve the first batch so f=0's weights don't
                            # wait on all four f-tiles
                            eng.dma_start(wst[:, :2], w1d[:, 0:2])
                            eng.dma_start(wst[:, 2:], w1d[:, 2:4])
                        else:
                            eng.dma_start(wst[:], w1d[:, f : f + 4])
                    if ci == 0 and f in (8, 14, 20, 26):
                        # W2-side: only the four PLAIN quarters stream (8.4MB,
                        # not 14.7MB of combos); B12 stages in T1's slot and
                        # the idle DVE forms T1..T4 during this GEMM1.  They
                        # ride gpsimd behind b0; their deadlines stretch into
                        # the first GEMM2 product sweep.
                        dst, src = {
                            8: (w2T[:, 0], w2xd[:]),      # B12
                            14: (w2q[:, 0], w2qd[:, 0]),  # B11
                            20: (w2q[:, 2], w2qd[:, 2]),  # B22
                            26: (w2q[:, 1], w2qd[:, 1]),  # B21
                        }[f]
                        nc.gpsimd.dma_start(dst, src)
                    if ci == 0 and f == 27:
                        # T3 = B22-B12 must read B12 before T1 overwrites it
                        # in place; both depend only on loads done by now.
                        nc.vector.tensor_tensor(w2T[:, 2], w2q[:, 2], w2T[:, 0], SUB)
                        nc.vector.tensor_tensor(w2T[:, 0], w2T[:, 0], w2q[:, 0], SUB)
                    pt1 = pp.tile([P, 512], F32, tag="pp")
                    for k in range(KH):
                        nc.tensor.matmul(
                            pt1[:],
                            wst[:, f % 4, k, :],
                            xt[:, k, :],
                            start=(k == 0),
                            stop=(k == KH - 1),
                        )
                    bias = b1t[:, f : f + 1] if use_b1 else 0.0
                    nc.scalar.activation(hq[:, f, :], pt1[:], gelu, bias=bias)
                    if f >= FH:
                        # U-trickle for j = f-16: h f-tiles j and 16+j are
                        # both in hq now, so this k2'-slice of the four
                        # combos can be formed while GEMM1 continues.
                        j = f - FH
                        a11 = hq[:, j, 0:256]
                        a21 = hq[:, j, 256:512]
                        a12 = hq[:, j + FH, 0:256]
                        a22 = hq[:, j + FH, 256:512]
                        v = nc.vector
                        v.tensor_tensor(ut[:, 0, j], a21, a22, ADD)
                        v.tensor_tensor(ut[:, 1, j], ut[:, 0, j], a11, SUB)
                        v.tensor_tensor(ut[:, 2, j], a11, a21, SUB)
                        v.tensor_tensor(ut[:, 3, j], a12, ut[:, 1, j], SUB)

                if ci == 0:
                    # T2 = B22-T1, T4 = T2-B21: issued after the U-trickle so
                    # they don't head-block it on the vector engine.
                    nc.vector.tensor_tensor(w2T[:, 1], w2q[:, 2], w2T[:, 0], SUB)
                    nc.vector.tensor_tensor(w2T[:, 3], w2T[:, 1], w2q[:, 1], SUB)
                # next chunk's x tile: the single x buffer is idle during
                # GEMM2 (only GEMM1 reads it), so prefetch into it now.
                if ci + 1 < n512:
                    xtn = xp.tile([P, KH, 512], BF16, tag="xt")
                    nc.gpsimd.dma_start(xtn[:], xda[:, ci + 1])

                # ---- GEMM2, one Winograd-Strassen level ----
                # products M1..M7 are [256, 512] over the M-half row
                # space; product row r of A2x combos is token 256+r.
                for tt in range(2):
                    ts0 = slice(tt * P, (tt + 1) * P)
                    ts2 = slice(256 + tt * P, 256 + (tt + 1) * P)
                    lhs = (
                        lambda j: ut[:, 1, j, ts0],    # M1: S2
                        lambda j: hq[:, j, ts0],       # M2: A11
                        lambda j: hq[:, j + FH, ts0],  # M3: A12
                        lambda j: ut[:, 2, j, ts0],    # M4: S3
                        lambda j: ut[:, 0, j, ts0],    # M5: S1
                        lambda j: ut[:, 3, j, ts0],    # M6: S4
                        lambda j: hq[:, j + FH, ts2],  # M7: A22
                    )
                    mov = (
                        w2T[:, 1],  # M1: T2
                        w2q[:, 0],  # M2: B11
                        w2q[:, 1],  # M3: B21
                        w2T[:, 2],  # M4: T3
                        w2T[:, 0],  # M5: T1
                        w2q[:, 2],  # M6: B22
                        w2T[:, 3],  # M7: T4
                    )
                    pr = {}
                    last = ci == n512 - 1 and tt == 1
                    order = (1, 2, 0, 3, 4, 6, 5) if last else (1, 2, 0, 3, 4, 5, 6)
                    for i in order:
                        pt = pp.tile([P, 512], F32, tag="pp")
                        for j in range(FH):
                            nc.tensor.matmul(
                                pt[:],
                                lhs[i](j),
                                mov[i][:, j],
                                start=(j == 0),
                                stop=(j == FH - 1),
                            )
                        pr[i] = pt[:]
                    m1, m2, m3, m4, m5, m6, m7 = (pr[i] for i in range(7))

                    # C-recombination:
                    #   u2 = M1+M2; C11 = M2+M3; u3 = u2+M4
                    #   C12 = u2+M5+M6; C21 = u3-M7; C22 = u3+M5
                    u2 = srp.tile([P, 512], F32, tag="sr")
                    u3 = srp.tile([P, 512], F32, tag="sr")
                    c3 = op.tile([P, 512], BF16, tag="ot")
                    o11 = op.tile([P, 512], BF16, tag="ot")
                    o12 = op.tile([P, 512], BF16, tag="ot")
                    o21 = op.tile([P, 512], BF16, tag="ot")
                    o22 = op.tile([P, 512], BF16, tag="ot")
                    v = nc.vector
                    nc.scalar.activation(c3[:], m3, ident)
                    v.tensor_tensor(o11[:], c3[:], m2, ADD)
                    nc.scalar.activation(u2[:], m1, ident)
                    v.tensor_tensor(u2[:], u2[:], m2, ADD)
                    v.tensor_tensor(u3[:], u2[:], m4, ADD)
                    v.tensor_tensor(u2[:], u2[:], m5, ADD)
                    v.tensor_tensor(o12[:], u2[:], m6, ADD)
                    v.tensor_tensor(o21[:], u3[:], m7, SUB)
                    v.tensor_tensor(o22[:], u3[:], m5, ADD)

                    trow = coff // P
                    nc.sync.dma_start(yd[:, trow + tt, 0:512], o11[:])
                    nc.sync.dma_start(yd[:, trow + tt, 512:1024], o12[:])
                    nc.sync.dma_start(yd[:, trow + 2 + tt, 0:512], o21[:])
                    nc.sync.dma_start(yd[:, trow + 2 + tt, 512:1024], o22[:])
                coff += 512

    nc.compile()
    return nc


def _gelu_tanh(v):
    # jax.nn.gelu(approximate=True): 0.5x(1+tanh(sqrt(2/pi)(x+0.044715x^3)))
    return 0.5 * v * (1.0 + np.tanh(0.7978845608028654 * (v + 0.044715 * v**3)))


def _route(x2d, Wg):
    """Replicates reference router: softmax -> top-2 -> renormalize."""
    logits = x2d @ Wg  # [T, E] fp32
    m = logits.max(axis=-1, keepdims=True)
    p = np.exp(logits - m, dtype=np.float32)
    p /= p.sum(axis=-1, keepdims=True)
    # jax.lax.top_k: values descending, ties broken by lower index.
    order = np.argsort(-p, axis=-1, kind="stable")
    top_i = order[:, :TOP_K]  # [T, 2]
    top_p = np.take_along_axis(p, top_i, axis=-1)
    top_p = top_p / top_p.sum(axis=-1, keepdims=True)
    return top_i, top_p


def kernel(x, Wg, W1, b1, W2, b2):
    global LAST_RESULT
    x = np.ascontiguousarray(np.asarray(x, dtype=np.float32))
    Wg = np.ascontiguousarray(np.asarray(Wg, dtype=np.float32))
    W1 = np.ascontiguousarray(np.asarray(W1, dtype=np.float32))
    b1 = np.ascontiguousarray(np.asarray(b1, dtype=np.float32))
    W2 = np.ascontiguousarray(np.asarray(W2, dtype=np.float32))
    b2 = np.ascontiguousarray(np.asarray(b2, dtype=np.float32))

    x2d = x.reshape(T, H)
    top_i, top_p = _route(x2d, Wg)

    rows = [None] * E
    gval = [None] * E
    for e in range(E):
        r, slot = np.nonzero(top_i == e)
        rows[e] = r
        gval[e] = top_p[r, slot]

    # Expert capacity (factor 1.0): each core computes at most T*K/E =
    # 2048 token slots -- the perfectly balanced load.  The few overflow
    # tokens of over-subscribed experts (~1.8% of assignments for this
    # routing) are evaluated in fp32 during the host-side combine below,
    # exactly like the router and gate application already are.
    cap = T * TOP_K // E
    c_max = max(len(r) for r in rows)
    c_pad = max(512, min(c_max, cap))
    c_pad = -(-c_pad // 512) * 512  # Winograd GEMM2 wants full 512 chunks
    n512 = c_pad // 512
    nrows = c_pad // P
    use_b1 = bool(np.any(b1))

    key = (n512, use_b1)
    if key not in _CACHE:
        _CACHE[key] = _build(n512, use_b1)
    nc = _CACHE[key]

    np_bf16 = mybir.dt.np(BF16)
    in_maps = []
    for e in range(E):
        cd = min(len(rows[e]), c_pad)
        xt = np.zeros((H, c_pad), np.float32)
        xt[:, :cd] = x2d[rows[e][:cd]].T
        # [P, KH, c_pad] view, then chunk-major repack
        xpkh = xt.reshape(KH, P, c_pad).transpose(1, 0, 2).astype(np_bf16)
        xa = np.ascontiguousarray(
            xpkh.reshape(P, KH, n512, 512).transpose(0, 2, 1, 3)
        )
        # Winograd moving-side W2 operands (quarters + T-combos):
        w2e = W2[e]
        B11 = w2e[:2048, :512]
        B12 = w2e[:2048, 512:]
        B21 = w2e[2048:, :512]
        B22 = w2e[2048:, 512:]
        Qq = np.stack([B11, B21, B22])  # [3, 2048, 512]
        m = {
            "xda": xa,
            "w1d": np.ascontiguousarray(
                W1[e].reshape(KH, P, FT, P).transpose(1, 2, 0, 3).astype(np_bf16)
            ),
            "w2qd": np.ascontiguousarray(
                Qq.reshape(3, FH, P, 512).transpose(2, 0, 1, 3).astype(np_bf16)
            ),
            "w2xd": np.ascontiguousarray(
                B12.reshape(FH, P, 512).transpose(1, 0, 2).astype(np_bf16)
            ),
        }
        if use_b1:
            m["b1d"] = np.ascontiguousarray(b1[e].reshape(FT, P).T)
        in_maps.append(m)

    trace = os.environ.get("KERNEL_TRACE", "") == "1"
    res = run_bass_kernel_spmd(
        nc,
        in_maps,
        core_ids=list(range(N_CORES)),
        trace=trace,
        trace_cores=[0] if trace else None,
    )
    LAST_RESULT = res

    out = np.zeros((T, H), np.float32)
    for e in range(E):
        cd = min(len(rows[e]), c_pad)
        yt = res.results[e]["yd"].astype(np.float32)  # [P, nrows, H]
        y = yt.transpose(1, 0, 2).reshape(nrows * P, H)[:cd]
        out[rows[e][:cd]] += gval[e][:cd, None] * (y + b2[e][None, :])
        if len(rows[e]) > cd:  # capacity overflow: fp32 on host
            ro = rows[e][cd:]
            ho = _gelu_tanh(x2d[ro] @ W1[e] + b1[e][None, :])
            yo = ho @ W2[e] + b2[e][None, :]
            out[ro] += gval[e][cd:, None] * yo

    return out.reshape(B, S, H)


# revision 42
# speedup vs baseline: 1.0338x; 1.0338x over previous
"""MoE FFN (top-2 of 8 experts) Trainium2 kernel.

Strategy (expert-parallel across 8 NeuronCores):
  - Host computes the (tiny) router: logits = x@Wg, softmax, top-2,
    renormalized combine weights.  Tokens are gathered per expert on the
    host ("all-to-all dispatch" done at sharding time), transposed to
    [H, C] so both FFN GEMMs run with natural weight layouts on device.
  - Core e runs the FFN for expert e over its C_pad gathered tokens.
  - GEMM1 is dense: plain W1 f-slices stream from HBM in 4-f-tile
    batches, alternating between the two DMA queues (one queue alone
    peaks well under the ~320 GB/s per-core cap).
  - GEMM2 uses one level of the Winograd variant of Strassen (7
    products, 15 additions) over the 2x2x2 split of [Nt,F]x[F,H]: the
    seven moving-side W2 operands (B11, B21, B22, T1..T4) are host-
    precomputed, loaded ONCE at the head in first-use order, and stay
    resident (112 KiB/partition); the four stationary-side h combos
    (S1..S4) trickle out on the vector engine as GEMM1 activations
    land (no pipeline seam); the C-block recombination runs as 7
    vector-engine ops (one PSUM operand each) plus 2 scalar Identity
    copies per token-tile.  Cuts GEMM2's PE row count by 12.5%.
  - Host applies combine weights + b2 and scatter-adds back ("combine").

The kernel is compiled once per (C_pad, biases-zero) configuration and
cached in-process; C_pad is rounded up to a multiple of 512.
"""

import os
import sys
import numpy as np

for _p in ("/opt/trn_rl_repo", "/root/.axon_site/_ro/trn_rl_repo"):
    if _p not in sys.path and os.path.isdir(_p):
        sys.path.append(_p)

import concourse.bacc as bacc  # noqa: E402
import concourse.tile as tile  # noqa: E402
from concourse import mybir  # noqa: E402
from concourse.bass_utils import run_bass_kernel_spmd  # noqa: E402

# Problem shapes (hardcoded per spec)
B, S, H, F, E = 4, 2048, 1024, 4096, 8
T = B * S
TOP_K = 2
N_CORES = 8
P = 128
KH = H // P          # 8  H-contraction subtiles
FT = F // P          # 32 f-tiles total
FH = FT // 2         # 16 f-tiles per Winograd half

F32 = mybir.dt.float32
BF16 = mybir.dt.bfloat16
ADD = mybir.AluOpType.add
SUB = mybir.AluOpType.subtract

_CACHE: dict = {}
LAST_RESULT = None  # BassKernelResults of the most recent run (for test.py)


def _build(n512: int, use_b1: bool):
    nrows = n512 * 4

    nc = bacc.Bacc(
        "TRN2",
        target_bir_lowering=False,
        debug=False,
        enable_asserts=False,
        num_devices=N_CORES,
    )

    # x staged chunk-major: each chunk is contiguous per partition.
    xda = nc.dram_tensor("xda", [P, n512, KH, 512], BF16, kind="ExternalInput").ap()
    # plain W1 tiles, streamed per 4-f-slice batch for the dense GEMM1
    w1d = nc.dram_tensor("w1d", [P, FT, KH, P], BF16, kind="ExternalInput").ap()
    # Winograd moving-side W2 operands: plain quarters and T-combos.
    # w2qd ops = (B11, B21, B22); w2Td ops = (T1, T2, T3, T4), each
    # [p, op, k2', n] with k2' the F-subtile within the F-half.
    w2qd = nc.dram_tensor("w2qd", [P, 3, FH, 512], BF16, kind="ExternalInput").ap()
    w2xd = nc.dram_tensor("w2xd", [P, FH, 512], BF16, kind="ExternalInput").ap()
    if use_b1:
        b1d = nc.dram_tensor("b1d", [P, FT], F32, kind="ExternalInput").ap()
    yd = nc.dram_tensor("yd", [P, nrows, H], BF16, kind="ExternalOutput").ap()

    gelu = mybir.ActivationFunctionType.Gelu_apprx_tanh
    ident = mybir.ActivationFunctionType.Identity

    with tile.TileContext(nc) as tc:
        with (
            tc.tile_pool(name="xp", bufs=1) as xp,
            tc.tile_pool(name="w1sp", bufs=2) as w1sp,
            tc.tile_pool(name="hp", bufs=1) as hp,
            tc.tile_pool(name="up", bufs=1) as up,
            tc.tile_pool(name="w2qp", bufs=1) as w2qp,
            tc.tile_pool(name="w2Tp", bufs=1) as w2Tp,
            tc.tile_pool(name="srp", bufs=2) as srp,
            tc.tile_pool(name="op", bufs=3) as op,
            tc.tile_pool(name="bp", bufs=1) as bp,
            tc.tile_pool(name="pp", bufs=8, space="PSUM") as pp,
        ):
            if use_b1:
                b1t = bp.tile([P, FT], F32)
                nc.sync.dma_start(b1t[:], b1d[:])

            # HAM warmup: the PE clock-gate sits at 1.2 GHz until it sees
            # ~3.4us of sustained matmul activity.  The PE is idle anyway
            # while the first operands stream in; burn that window on junk
            # N=64 matmuls so the real stream starts at the full 2.4 GHz.
            wub = op.tile([P, 512], BF16, tag="ot")
            nc.gpsimd.memset(wub[:, :P], 0)
            wups = pp.tile([P, 512], F32, tag="pp")
            for _ in range(170):
                nc.tensor.matmul(wups[:, :64], wub[:, :P], wub[:, :64], start=True, stop=True)

            # Chunk-0 x at the FRONT of the sync ring, then the W2-side
            # Winograd operands in first-use order (their deadlines are
            # staggered through the first chunk's GEMM2 product sweep).
            xt0 = xp.tile([P, KH, 512], BF16, tag="xt")
            nc.sync.dma_start(xt0[:, :4], xda[:, 0, :4])
            nc.sync.dma_start(xt0[:, 4:], xda[:, 0, 4:])

            w2q = w2qp.tile([P, 3, FH, 512], BF16)
            w2T = w2Tp.tile([P, 4, FH, 512], BF16)

            # h-side Winograd combos, trickled during GEMM1:
            #   U1=A21+A22, U2=U1-A11, U3=A11-A21, U4=A12-U2
            ut = up.tile([P, 4, FH, 256], BF16)

            coff = 0
            xtn = None
            for ci in range(n512):
                xt = xt0 if ci == 0 else xtn

                # ---- GEMM1, dense, W1 batches alternating DMA queues ----
                # Chunk 0 consumes its batches INTERLEAVED between the two
                # queues (gpsimd carries b0-b2, sync x0+b3-b7): processing
                # order 0-3, 12-15, 4-7, 16-19, 8-11, 20-31 gives every
                # batch >=8us of arrival margin instead of a just-in-time
                # race on the slower-booting gpsimd queue.  The U-trickle
                # pair (f, f-16) stays valid: every f>=16 group comes after
                # its partner group in this order.
                if ci == 0:
                    f_list = [0, 1, 2, 3, 12, 13, 14, 15, 4, 5, 6, 7,
                              16, 17, 18, 19, 8, 9, 10, 11] + list(range(20, FT))
                else:
                    f_list = list(range(FT))
                hq = hp.tile([P, FT, 512], BF16, tag="hq")
                for idx, f in enumerate(f_list):
                    if idx % 4 == 0:
                        wst = w1sp.tile([P, 4, KH, P], BF16, tag="wst")
                        if ci == 0:
                            eng = nc.gpsimd if f <= 8 else nc.sync
                        else:
                            eng = nc.gpsimd if (f // 4) % 2 == 0 else nc.sync
                        if ci == 0 and f == 0:
                            # halve the first batch so f=0's weights don't
                            # wait on all four f-tiles
                            eng.dma_start(wst[:, :2], w1d[:, 0:2])
                            eng.dma_start(wst[:, 2:], w1d[:, 2:4])
                        else:
                            eng.dma_start(wst[:], w1d[:, f : f + 4])
                        if ci == 0 and f == 8:
                            # W2-side: only the four PLAIN quarters stream
                            # (8.4MB, not 14.7MB of combos); B12 stages in
                            # T1's slot and the idle DVE forms T1..T4 during
                            # this GEMM1.  They ride gpsimd behind its last
                            # W1 batch; deadlines stretch into GEMM2.
                            nc.gpsimd.dma_start(w2T[:, 0], w2xd[:])     # B12
                            nc.gpsimd.dma_start(w2q[:, 0], w2qd[:, 0])  # B11
                            nc.gpsimd.dma_start(w2q[:, 2], w2qd[:, 2])  # B22
                            nc.gpsimd.dma_start(w2q[:, 1], w2qd[:, 1])  # B21
                    if ci == 0 and idx == 27:
                        # T3 = B22-B12 must read B12 before T1 overwrites it
                        # in place; both depend only on loads done by now.
                        nc.vector.tensor_tensor(w2T[:, 2], w2q[:, 2], w2T[:, 0], SUB)
                        nc.vector.tensor_tensor(w2T[:, 0], w2T[:, 0], w2q[:, 0], SUB)
                    pt1 = pp.tile([P, 512], F32, tag="pp")
                    for k in range(KH):
                        nc.tensor.matmul(
                            pt1[:],
                            wst[:, f % 4, k, :],
                            xt[:, k, :],
                            start=(k == 0),
                            stop=(k == KH - 1),
                        )
                    bias = b1t[:, f : f + 1] if use_b1 else 0.0
                    nc.scalar.activation(hq[:, f, :], pt1[:], gelu, bias=bias)
                    if f >= FH:
                        # U-trickle for j = f-16: h f-tiles j and 16+j are
                        # both in hq now, so this k2'-slice of the four
                        # combos can be formed while GEMM1 continues.
                        j = f - FH
                        a11 = hq[:, j, 0:256]
                        a21 = hq[:, j, 256:512]
                        a12 = hq[:, j + FH, 0:256]
                        a22 = hq[:, j + FH, 256:512]
                        v = nc.vector
                        v.tensor_tensor(ut[:, 0, j], a21, a22, ADD)
                        v.tensor_tensor(ut[:, 1, j], ut[:, 0, j], a11, SUB)
                        v.tensor_tensor(ut[:, 2, j], a11, a21, SUB)
                        v.tensor_tensor(ut[:, 3, j], a12, ut[:, 1, j], SUB)

                if ci == 0:
                    # T2 = B22-T1, T4 = T2-B21: issued after the U-trickle so
                    # they don't head-block it on the vector engine.
                    nc.vector.tensor_tensor(w2T[:, 1], w2q[:, 2], w2T[:, 0], SUB)
                    nc.vector.tensor_tensor(w2T[:, 3], w2T[:, 1], w2q[:, 1], SUB)
                # next chunk's x tile: the single x buffer is idle during
                # GEMM2 (only GEMM1 reads it), so prefetch into it now.
                if ci + 1 < n512:
                    xtn = xp.tile([P, KH, 512], BF16, tag="xt")
                    nc.gpsimd.dma_start(xtn[:], xda[:, ci + 1])

                # ---- GEMM2, one Winograd-Strassen level ----
                # products M1..M7 are [256, 512] over the M-half row
                # space; product row r of A2x combos is token 256+r.
                for tt in range(2):
                    ts0 = slice(tt * P, (tt + 1) * P)
                    ts2 = slice(256 + tt * P, 256 + (tt + 1) * P)
                    lhs = (
                        lambda j: ut[:, 1, j, ts0],    # M1: S2
                        lambda j: hq[:, j, ts0],       # M2: A11
                        lambda j: hq[:, j + FH, ts0],  # M3: A12
                        lambda j: ut[:, 2, j, ts0],    # M4: S3
                        lambda j: ut[:, 0, j, ts0],    # M5: S1
                        lambda j: ut[:, 3, j, ts0],    # M6: S4
                        lambda j: hq[:, j + FH, ts2],  # M7: A22
                    )
                    mov = (
                        w2T[:, 1],  # M1: T2
                        w2q[:, 0],  # M2: B11
                        w2q[:, 1],  # M3: B21
                        w2T[:, 2],  # M4: T3
                        w2T[:, 0],  # M5: T1
                        w2q[:, 2],  # M6: B22
                        w2T[:, 3],  # M7: T4
                    )
                    pr = {}
                    last = ci == n512 - 1 and tt == 1
                    order = (1, 2, 0, 3, 4, 6, 5) if last else (1, 2, 0, 3, 4, 5, 6)
                    for i in order:
                        pt = pp.tile([P, 512], F32, tag="pp")
                        for j in range(FH):
                            nc.tensor.matmul(
                                pt[:],
                                lhs[i](j),
                                mov[i][:, j],
                                start=(j == 0),
                                stop=(j == FH - 1),
                            )
                        pr[i] = pt[:]
                    m1, m2, m3, m4, m5, m6, m7 = (pr[i] for i in range(7))

                    # C-recombination:
                    #   u2 = M1+M2; C11 = M2+M3; u3 = u2+M4
                    #   C12 = u2+M5+M6; C21 = u3-M7; C22 = u3+M5
                    u2 = srp.tile([P, 512], F32, tag="sr")
                    u3 = srp.tile([P, 512], F32, tag="sr")
                    c3 = op.tile([P, 512], BF16, tag="ot")
                    o11 = op.tile([P, 512], BF16, tag="ot")
                    o12 = op.tile([P, 512], BF16, tag="ot")
                    o21 = op.tile([P, 512], BF16, tag="ot")
                    o22 = op.tile([P, 512], BF16, tag="ot")
                    v = nc.vector
                    nc.scalar.activation(c3[:], m3, ident)
                    v.tensor_tensor(o11[:], c3[:], m2, ADD)
                    nc.scalar.activation(u2[:], m1, ident)
                    v.tensor_tensor(u2[:], u2[:], m2, ADD)
                    v.tensor_tensor(u3[:], u2[:], m4, ADD)
                    v.tensor_tensor(u2[:], u2[:], m5, ADD)
                    v.tensor_tensor(o12[:], u2[:], m6, ADD)
                    v.tensor_tensor(o21[:], u3[:], m7, SUB)
                    v.tensor_tensor(o22[:], u3[:], m5, ADD)

                    trow = coff // P
                    nc.sync.dma_start(yd[:, trow + tt, 0:512], o11[:])
                    nc.sync.dma_start(yd[:, trow + tt, 512:1024], o12[:])
                    nc.sync.dma_start(yd[:, trow + 2 + tt, 0:512], o21[:])
                    nc.sync.dma_start(yd[:, trow + 2 + tt, 512:1024], o22[:])
                coff += 512

    nc.compile()
    return nc


def _gelu_tanh(v):
    # jax.nn.gelu(approximate=True): 0.5x(1+tanh(sqrt(2/pi)(x+0.044715x^3)))
    return 0.5 * v * (1.0 + np.tanh(0.7978845608028654 * (v + 0.044715 * v**3)))


def _route(x2d, Wg):
    """Replicates reference router: softmax -> top-2 -> renormalize."""
    logits = x2d @ Wg  # [T, E] fp32
    m = logits.max(axis=-1, keepdims=True)
    p = np.exp(logits - m, dtype=np.float32)
    p /= p.sum(axis=-1, keepdims=True)
    # jax.lax.top_k: values descending, ties broken by lower index.
    order = np.argsort(-p, axis=-1, kind="stable")
    top_i = order[:, :TOP_K]  # [T, 2]
    top_p = np.take_along_axis(p, top_i, axis=-1)
    top_p = top_p / top_p.sum(axis=-1, keepdims=True)
    return top_i, top_p


def kernel(x, Wg, W1, b1, W2, b2):
    global LAST_RESULT
    x = np.ascontiguousarray(np.asarray(x, dtype=np.float32))
    Wg = np.ascontiguousarray(np.asarray(Wg, dtype=np.float32))
    W1 = np.ascontiguousarray(np.asarray(W1, dtype=np.float32))
    b1 = np.ascontiguousarray(np.asarray(b1, dtype=np.float32))
    W2 = np.ascontiguousarray(np.asarray(W2, dtype=np.float32))
    b2 = np.ascontiguousarray(np.asarray(b2, dtype=np.float32))

    x2d = x.reshape(T, H)
    top_i, top_p = _route(x2d, Wg)

    rows = [None] * E
    gval = [None] * E
    for e in range(E):
        r, slot = np.nonzero(top_i == e)
        rows[e] = r
        gval[e] = top_p[r, slot]

    # Expert capacity (factor 1.0): each core computes at most T*K/E =
    # 2048 token slots -- the perfectly balanced load.  The few overflow
    # tokens of over-subscribed experts (~1.8% of assignments for this
    # routing) are evaluated in fp32 during the host-side combine below,
    # exactly like the router and gate application already are.
    cap = T * TOP_K // E
    c_max = max(len(r) for r in rows)
    c_pad = max(512, min(c_max, cap))
    c_pad = -(-c_pad // 512) * 512  # Winograd GEMM2 wants full 512 chunks
    n512 = c_pad // 512
    nrows = c_pad // P
    use_b1 = bool(np.any(b1))

    key = (n512, use_b1)
    if key not in _CACHE:
        _CACHE[key] = _build(n512, use_b1)
    nc = _CACHE[key]

    np_bf16 = mybir.dt.np(BF16)
    in_maps = []
    for e in range(E):
        cd = min(len(rows[e]), c_pad)
        xt = np.zeros((H, c_pad), np.float32)
        xt[:, :cd] = x2d[rows[e][:cd]].T
        # [P, KH, c_pad] view, then chunk-major repack
        xpkh = xt.reshape(KH, P, c_pad).transpose(1, 0, 2).astype(np_bf16)
        xa = np.ascontiguousarray(
            xpkh.reshape(P, KH, n512, 512).transpose(0, 2, 1, 3)
        )
        # Winograd moving-side W2 operands (quarters + T-combos):
        w2e = W2[e]
        B11 = w2e[:2048, :512]
        B12 = w2e[:2048, 512:]
        B21 = w2e[2048:, :512]
        B22 = w2e[2048:, 512:]
        Qq = np.stack([B11, B21, B22])  # [3, 2048, 512]
        m = {
            "xda": xa,
            "w1d": np.ascontiguousarray(
                W1[e].reshape(KH, P, FT, P).transpose(1, 2, 0, 3).astype(np_bf16)
            ),
            "w2qd": np.ascontiguousarray(
                Qq.reshape(3, FH, P, 512).transpose(2, 0, 1, 3).astype(np_bf16)
            ),
            "w2xd": np.ascontiguousarray(
                B12.reshape(FH, P, 512).transpose(1, 0, 2).astype(np_bf16)
            ),
        }
        if use_b1:
            m["b1d"] = np.ascontiguousarray(b1[e].reshape(FT, P).T)
        in_maps.append(m)

    trace = os.environ.get("KERNEL_TRACE", "") == "1"
    res = run_bass_kernel_spmd(
        nc,
        in_maps,
        core_ids=list(range(N_CORES)),
        trace=trace,
        trace_cores=[0] if trace else None,
    )
    LAST_RESULT = res

    out = np.zeros((T, H), np.float32)
    for e in range(E):
        cd = min(len(rows[e]), c_pad)
        yt = res.results[e]["yd"].astype(np.float32)  # [P, nrows, H]
        y = yt.transpose(1, 0, 2).reshape(nrows * P, H)[:cd]
        out[rows[e][:cd]] += gval[e][:cd, None] * (y + b2[e][None, :])
        if len(rows[e]) > cd:  # capacity overflow: fp32 on host
            ro = rows[e][cd:]
            ho = _gelu_tanh(x2d[ro] @ W1[e] + b1[e][None, :])
            yo = ho @ W2[e] + b2[e][None, :]
            out[ro] += gval[e][cd:, None] * yo

    return out.reshape(B, S, H)


# revision 43
# speedup vs baseline: 1.0368x; 1.0029x over previous
"""MoE FFN (top-2 of 8 experts) Trainium2 kernel.

Strategy (expert-parallel across 8 NeuronCores):
  - Host computes the (tiny) router: logits = x@Wg, softmax, top-2,
    renormalized combine weights.  Tokens are gathered per expert on the
    host ("all-to-all dispatch" done at sharding time), transposed to
    [H, C] so both FFN GEMMs run with natural weight layouts on device.
  - Core e runs the FFN for expert e over its C_pad gathered tokens.
  - GEMM1 is dense: plain W1 f-slices stream from HBM in 4-f-tile
    batches, alternating between the two DMA queues (one queue alone
    peaks well under the ~320 GB/s per-core cap).
  - GEMM2 uses one level of the Winograd variant of Strassen (7
    products, 15 additions) over the 2x2x2 split of [Nt,F]x[F,H]: the
    seven moving-side W2 operands (B11, B21, B22, T1..T4) are host-
    precomputed, loaded ONCE at the head in first-use order, and stay
    resident (112 KiB/partition); the four stationary-side h combos
    (S1..S4) trickle out on the vector engine as GEMM1 activations
    land (no pipeline seam); the C-block recombination runs as 7
    vector-engine ops (one PSUM operand each) plus 2 scalar Identity
    copies per token-tile.  Cuts GEMM2's PE row count by 12.5%.
  - Host applies combine weights + b2 and scatter-adds back ("combine").

The kernel is compiled once per (C_pad, biases-zero) configuration and
cached in-process; C_pad is rounded up to a multiple of 512.
"""

import os
import sys
import numpy as np

for _p in ("/opt/trn_rl_repo", "/root/.axon_site/_ro/trn_rl_repo"):
    if _p not in sys.path and os.path.isdir(_p):
        sys.path.append(_p)

import concourse.bacc as bacc  # noqa: E402
import concourse.tile as tile  # noqa: E402
from concourse import mybir  # noqa: E402
from concourse.bass_utils import run_bass_kernel_spmd  # noqa: E402

# Problem shapes (hardcoded per spec)
B, S, H, F, E = 4, 2048, 1024, 4096, 8
T = B * S
TOP_K = 2
N_CORES = 8
P = 128
KH = H // P          # 8  H-contraction subtiles
FT = F // P          # 32 f-tiles total
FH = FT // 2         # 16 f-tiles per Winograd half

F32 = mybir.dt.float32
BF16 = mybir.dt.bfloat16
ADD = mybir.AluOpType.add
SUB = mybir.AluOpType.subtract

_CACHE: dict = {}
LAST_RESULT = None  # BassKernelResults of the most recent run (for test.py)


def _build(n512: int, use_b1: bool):
    nrows = n512 * 4

    nc = bacc.Bacc(
        "TRN2",
        target_bir_lowering=False,
        debug=False,
        enable_asserts=False,
        num_devices=N_CORES,
    )

    # x staged chunk-major: each chunk is contiguous per partition.
    xda = nc.dram_tensor("xda", [P, n512, KH, 512], BF16, kind="ExternalInput").ap()
    # plain W1 tiles, streamed per 4-f-slice batch for the dense GEMM1
    w1d = nc.dram_tensor("w1d", [P, FT, KH, P], BF16, kind="ExternalInput").ap()
    # Winograd moving-side W2 operands: plain quarters and T-combos.
    # w2qd ops = (B11, B21, B22); w2Td ops = (T1, T2, T3, T4), each
    # [p, op, k2', n] with k2' the F-subtile within the F-half.
    w2qd = nc.dram_tensor("w2qd", [P, 3, FH, 512], BF16, kind="ExternalInput").ap()
    w2xd = nc.dram_tensor("w2xd", [P, FH, 512], BF16, kind="ExternalInput").ap()
    if use_b1:
        b1d = nc.dram_tensor("b1d", [P, FT], F32, kind="ExternalInput").ap()
    yd = nc.dram_tensor("yd", [P, nrows, H], BF16, kind="ExternalOutput").ap()

    gelu = mybir.ActivationFunctionType.Gelu_apprx_tanh
    ident = mybir.ActivationFunctionType.Identity

    with tile.TileContext(nc) as tc:
        with (
            tc.tile_pool(name="xp", bufs=1) as xp,
            tc.tile_pool(name="w1sp", bufs=2) as w1sp,
            tc.tile_pool(name="hp", bufs=1) as hp,
            tc.tile_pool(name="up", bufs=1) as up,
            tc.tile_pool(name="w2qp", bufs=1) as w2qp,
            tc.tile_pool(name="w2Tp", bufs=1) as w2Tp,
            tc.tile_pool(name="srp", bufs=2) as srp,
            tc.tile_pool(name="op", bufs=3) as op,
            tc.tile_pool(name="bp", bufs=1) as bp,
            tc.tile_pool(name="pp", bufs=8, space="PSUM") as pp,
        ):
            if use_b1:
                b1t = bp.tile([P, FT], F32)
                nc.sync.dma_start(b1t[:], b1d[:])

            # HAM warmup: the PE clock-gate sits at 1.2 GHz until it sees
            # ~3.4us of sustained matmul activity.  The PE is idle anyway
            # while the first operands stream in; burn that window on junk
            # N=64 matmuls so the real stream starts at the full 2.4 GHz.
            wub = op.tile([P, 512], BF16, tag="ot")
            nc.gpsimd.memset(wub[:, :P], 0)
            wups = pp.tile([P, 512], F32, tag="pp")
            for _ in range(170):
                nc.tensor.matmul(wups[:, :64], wub[:, :P], wub[:, :64], start=True, stop=True)

            # Chunk-0 x at the FRONT of the sync ring, then the W2-side
            # Winograd operands in first-use order (their deadlines are
            # staggered through the first chunk's GEMM2 product sweep).
            xt0 = xp.tile([P, KH, 512], BF16, tag="xt")
            nc.sync.dma_start(xt0[:, :4], xda[:, 0, :4])
            nc.sync.dma_start(xt0[:, 4:], xda[:, 0, 4:])

            w2q = w2qp.tile([P, 3, FH, 512], BF16)
            w2T = w2Tp.tile([P, 4, FH, 512], BF16)

            # h-side Winograd combos, trickled during GEMM1:
            #   U1=A21+A22, U2=U1-A11, U3=A11-A21, U4=A12-U2
            ut = up.tile([P, 4, FH, 256], BF16)

            coff = 0
            xtn = None
            for ci in range(n512):
                xt = xt0 if ci == 0 else xtn

                # ---- GEMM1, dense, W1 batches alternating DMA queues ----
                hq = hp.tile([P, FT, 512], BF16, tag="hq")
                for f in range(FT):
                    if f % 4 == 0:
                        wst = w1sp.tile([P, 4, KH, P], BF16, tag="wst")
                        if ci == 0:
                            eng = nc.gpsimd if f <= 8 else nc.sync
                        else:
                            eng = nc.gpsimd if (f // 4) % 2 == 0 else nc.sync
                        if ci == 0 and f == 0:
                            # halve the first batch so f=0's weights don't
                            # wait on all four f-tiles
                            eng.dma_start(wst[:, :2], w1d[:, 0:2])
                            eng.dma_start(wst[:, 2:], w1d[:, 2:4])
                        else:
                            eng.dma_start(wst[:], w1d[:, f : f + 4])
                    if ci == 0 and f in (8, 14, 20, 26):
                        # W2-side: only the four PLAIN quarters stream (8.4MB,
                        # not 14.7MB of combos); B12 stages in T1's slot and
                        # the idle DVE forms T1..T4 during this GEMM1.  They
                        # ride gpsimd behind b0; their deadlines stretch into
                        # the first GEMM2 product sweep.
                        dst, src = {
                            8: (w2T[:, 0], w2xd[:]),      # B12
                            14: (w2q[:, 0], w2qd[:, 0]),  # B11
                            20: (w2q[:, 2], w2qd[:, 2]),  # B22
                            26: (w2q[:, 1], w2qd[:, 1]),  # B21
                        }[f]
                        nc.gpsimd.dma_start(dst, src)
                    if ci == 0 and f == 27:
                        # T3 = B22-B12 must read B12 before T1 overwrites it
                        # in place; both depend only on loads done by now.
                        nc.vector.tensor_tensor(w2T[:, 2], w2q[:, 2], w2T[:, 0], SUB)
                        nc.vector.tensor_tensor(w2T[:, 0], w2T[:, 0], w2q[:, 0], SUB)
                    pt1 = pp.tile([P, 512], F32, tag="pp")
                    for k in range(KH):
                        nc.tensor.matmul(
                            pt1[:],
                            wst[:, f % 4, k, :],
                            xt[:, k, :],
                            start=(k == 0),
                            stop=(k == KH - 1),
                        )
                    bias = b1t[:, f : f + 1] if use_b1 else 0.0
                    nc.scalar.activation(hq[:, f, :], pt1[:], gelu, bias=bias)
                    if f >= FH:
                        # U-trickle for j = f-16: h f-tiles j and 16+j are
                        # both in hq now, so this k2'-slice of the four
                        # combos can be formed while GEMM1 continues.
                        j = f - FH
                        a11 = hq[:, j, 0:256]
                        a21 = hq[:, j, 256:512]
                        a12 = hq[:, j + FH, 0:256]
                        a22 = hq[:, j + FH, 256:512]
                        v = nc.vector
                        v.tensor_tensor(ut[:, 0, j], a21, a22, ADD)
                        v.tensor_tensor(ut[:, 1, j], ut[:, 0, j], a11, SUB)
                        v.tensor_tensor(ut[:, 2, j], a11, a21, SUB)
                        v.tensor_tensor(ut[:, 3, j], a12, ut[:, 1, j], SUB)

                if ci == 0:
                    # T2 = B22-T1, T4 = T2-B21: issued after the U-trickle so
                    # they don't head-block it on the vector engine.
                    nc.vector.tensor_tensor(w2T[:, 1], w2q[:, 2], w2T[:, 0], SUB)
                    nc.vector.tensor_tensor(w2T[:, 3], w2T[:, 1], w2q[:, 1], SUB)
                # next chunk's x tile: the single x buffer is idle during
                # GEMM2 (only GEMM1 reads it), so prefetch into it now.
                if ci + 1 < n512:
                    xtn = xp.tile([P, KH, 512], BF16, tag="xt")
                    nc.gpsimd.dma_start(xtn[:], xda[:, ci + 1])

                # ---- GEMM2, one Winograd-Strassen level ----
                # products M1..M7 are [256, 512] over the M-half row
                # space; product row r of A2x combos is token 256+r.
                for tt in range(2):
                    ts0 = slice(tt * P, (tt + 1) * P)
                    ts2 = slice(256 + tt * P, 256 + (tt + 1) * P)
                    lhs = (
                        lambda j: ut[:, 1, j, ts0],    # M1: S2
                        lambda j: hq[:, j, ts0],       # M2: A11
                        lambda j: hq[:, j + FH, ts0],  # M3: A12
                        lambda j: ut[:, 2, j, ts0],    # M4: S3
                        lambda j: ut[:, 0, j, ts0],    # M5: S1
                        lambda j: ut[:, 3, j, ts0],    # M6: S4
                        lambda j: hq[:, j + FH, ts2],  # M7: A22
                    )
                    mov = (
                        w2T[:, 1],  # M1: T2
                        w2q[:, 0],  # M2: B11
                        w2q[:, 1],  # M3: B21
                        w2T[:, 2],  # M4: T3
                        w2T[:, 0],  # M5: T1
                        w2q[:, 2],  # M6: B22
                        w2T[:, 3],  # M7: T4
                    )
                    pr = {}
                    last = ci == n512 - 1 and tt == 1
                    order = (1, 2, 0, 3, 4, 6, 5) if last else (1, 2, 0, 3, 4, 5, 6)
                    for i in order:
                        pt = pp.tile([P, 512], F32, tag="pp")
                        for j in range(FH):
                            nc.tensor.matmul(
                                pt[:],
                                lhs[i](j),
                                mov[i][:, j],
                                start=(j == 0),
                                stop=(j == FH - 1),
                            )
                        pr[i] = pt[:]
                    m1, m2, m3, m4, m5, m6, m7 = (pr[i] for i in range(7))

                    # C-recombination:
                    #   u2 = M1+M2; C11 = M2+M3; u3 = u2+M4
                    #   C12 = u2+M5+M6; C21 = u3-M7; C22 = u3+M5
                    u2 = srp.tile([P, 512], F32, tag="sr")
                    u3 = srp.tile([P, 512], F32, tag="sr")
                    c3 = op.tile([P, 512], BF16, tag="ot")
                    o11 = op.tile([P, 512], BF16, tag="ot")
                    o12 = op.tile([P, 512], BF16, tag="ot")
                    o21 = op.tile([P, 512], BF16, tag="ot")
                    o22 = op.tile([P, 512], BF16, tag="ot")
                    v = nc.vector
                    nc.scalar.activation(c3[:], m3, ident)
                    v.tensor_tensor(o11[:], c3[:], m2, ADD)
                    nc.scalar.activation(u2[:], m1, ident)
                    v.tensor_tensor(u2[:], u2[:], m2, ADD)
                    v.tensor_tensor(u3[:], u2[:], m4, ADD)
                    v.tensor_tensor(u2[:], u2[:], m5, ADD)
                    v.tensor_tensor(o12[:], u2[:], m6, ADD)
                    v.tensor_tensor(o21[:], u3[:], m7, SUB)
                    v.tensor_tensor(o22[:], u3[:], m5, ADD)

                    trow = coff // P
                    nc.sync.dma_start(yd[:, trow + tt, 0:512], o11[:])
                    nc.sync.dma_start(yd[:, trow + tt, 512:1024], o12[:])
                    nc.sync.dma_start(yd[:, trow + 2 + tt, 0:512], o21[:])
                    nc.sync.dma_start(yd[:, trow + 2 + tt, 512:1024], o22[:])
                coff += 512

    nc.compile()
    return nc


def _gelu_tanh(v):
    # jax.nn.gelu(approximate=True): 0.5x(1+tanh(sqrt(2/pi)(x+0.044715x^3)))
    return 0.5 * v * (1.0 + np.tanh(0.7978845608028654 * (v + 0.044715 * v**3)))


def _route(x2d, Wg):
    """Replicates reference router: softmax -> top-2 -> renormalize."""
    logits = x2d @ Wg  # [T, E] fp32
    m = logits.max(axis=-1, keepdims=True)
    p = np.exp(logits - m, dtype=np.float32)
    p /= p.sum(axis=-1, keepdims=True)
    # jax.lax.top_k: values descending, ties broken by lower index.
    order = np.argsort(-p, axis=-1, kind="stable")
    top_i = order[:, :TOP_K]  # [T, 2]
    top_p = np.take_along_axis(p, top_i, axis=-1)
    top_p = top_p / top_p.sum(axis=-1, keepdims=True)
    return top_i, top_p


def kernel(x, Wg, W1, b1, W2, b2):
    global LAST_RESULT
    x = np.ascontiguousarray(np.asarray(x, dtype=np.float32))
    Wg = np.ascontiguousarray(np.asarray(Wg, dtype=np.float32))
    W1 = np.ascontiguousarray(np.asarray(W1, dtype=np.float32))
    b1 = np.ascontiguousarray(np.asarray(b1, dtype=np.float32))
    W2 = np.ascontiguousarray(np.asarray(W2, dtype=np.float32))
    b2 = np.ascontiguousarray(np.asarray(b2, dtype=np.float32))

    x2d = x.reshape(T, H)
    top_i, top_p = _route(x2d, Wg)

    rows = [None] * E
    gval = [None] * E
    for e in range(E):
        r, slot = np.nonzero(top_i == e)
        rows[e] = r
        gval[e] = top_p[r, slot]

    # Expert capacity (factor 1.0): each core computes at most T*K/E =
    # 2048 token slots -- the perfectly balanced load.  The few overflow
    # tokens of over-subscribed experts (~1.8% of assignments for this
    # routing) are evaluated in fp32 during the host-side combine below,
    # exactly like the router and gate application already are.
    cap = T * TOP_K // E
    c_max = max(len(r) for r in rows)
    c_pad = max(512, min(c_max, cap))
    c_pad = -(-c_pad // 512) * 512  # Winograd GEMM2 wants full 512 chunks
    n512 = c_pad // 512
    nrows = c_pad // P
    use_b1 = bool(np.any(b1))

    key = (n512, use_b1)
    if key not in _CACHE:
        _CACHE[key] = _build(n512, use_b1)
    nc = _CACHE[key]

    np_bf16 = mybir.dt.np(BF16)
    in_maps = []
    for e in range(E):
        cd = min(len(rows[e]), c_pad)
        xt = np.zeros((H, c_pad), np.float32)
        xt[:, :cd] = x2d[rows[e][:cd]].T
        # [P, KH, c_pad] view, then chunk-major repack
        xpkh = xt.reshape(KH, P, c_pad).transpose(1, 0, 2).astype(np_bf16)
        xa = np.ascontiguousarray(
            xpkh.reshape(P, KH, n512, 512).transpose(0, 2, 1, 3)
        )
        # Winograd moving-side W2 operands (quarters + T-combos):
        w2e = W2[e]
        B11 = w2e[:2048, :512]
        B12 = w2e[:2048, 512:]
        B21 = w2e[2048:, :512]
        B22 = w2e[2048:, 512:]
        Qq = np.stack([B11, B21, B22])  # [3, 2048, 512]
        m = {
            "xda": xa,
            "w1d": np.ascontiguousarray(
                W1[e].reshape(KH, P, FT, P).transpose(1, 2, 0, 3).astype(np_bf16)
            ),
            "w2qd": np.ascontiguousarray(
                Qq.reshape(3, FH, P, 512).transpose(2, 0, 1, 3).astype(np_bf16)
            ),
            "w2xd": np.ascontiguousarray(
                B12.reshape(FH, P, 512).transpose(1, 0, 2).astype(np_bf16)
            ),
        }
        if use_b1:
            m["b1d"] = np.ascontiguousarray(b1[e].reshape(FT, P).T)
        in_maps.append(m)

    trace = os.environ.get("KERNEL_TRACE", "") == "1"
    res = run_bass_kernel_spmd(
        nc,
        in_maps,
        core_ids=list(range(N_CORES)),
        trace=trace,
        trace_cores=[0] if trace else None,
    )
    LAST_RESULT = res

    out = np.zeros((T, H), np.float32)
    for e in range(E):
        cd = min(len(rows[e]), c_pad)
        yt = res.results[e]["yd"].astype(np.float32)  # [P, nrows, H]
        y = yt.transpose(1, 0, 2).reshape(nrows * P, H)[:cd]
        out[rows[e][:cd]] += gval[e][:cd, None] * (y + b2[e][None, :])
        if len(rows[e]) > cd:  # capacity overflow: fp32 on host
            ro = rows[e][cd:]
            ho = _gelu_tanh(x2d[ro] @ W1[e] + b1[e][None, :])
            yo = ho @ W2[e] + b2[e][None, :]
            out[ro] += gval[e][cd:, None] * yo

    return out.reshape(B, S, H)
